# revision 10
# baseline (speedup 1.0000x reference)
"""Multi-head attention (B=4, S=2048, D=1024, H=16, dk=dv=64) on 8 Trainium2
NeuronCores.

Sharding: core c handles batch b = c//2 and head-group g = c%2 (8 of 16 heads).

Host prep: inputs are transposed (x^T, [D, S]) and cast to bf16 on the host,
weights cast to bf16, so the device does no input transposes at all.

Per core:
  - K^T/Q^T projections [512, 2048] from bf16 x^T (moving) x weight tiles
    (stationary); biases folded into the PSUM->SBUF copies (alternating
    vector/scalar engines). V is projected directly to its natural layout
    [2048 s, 512 e] (x^T tiles as the stationary operand, Wv moving), bias
    added via a K=1 ones-row matmul; a ones column per head is appended so
    the PV matmul also produces softmax row-sums.
  - Attention per head-pair j, 512-wide query chunks: scores^T = K Q^T via
    row-tiled K=64 matmuls, exp on the scalar engine straight out of PSUM
    (mask as per-partition bias, 1/8 scale), bf16 probs, PV accumulated over
    16 key tiles.
  - Context (+rowsum row) is transposed back to natural, normalized with
    reciprocal rowsums, written out as `weights`, then transposed once more
    to head-dim-major (bf16) as the o_proj moving operand.
  - o_proj computed transposed: out^T[d, s] accumulating wo-tiles
    (stationary) x wts^T (moving); bo enters as a per-partition bias on the
    PSUM->SBUF copy (gated to g=0 cores). Host transposes back and sums the
    core pair (the row-parallel all-reduce).
  - Q-projection chunks for query block q+1 and all post-processing run as
    deferred ~1us work units popped inside later attention loops, filling
    the tensor-engine slack under the scalar-engine exp stream.
"""
import sys

for _p in ("/opt/trn_rl_repo", "/root/.axon_site/_ro/trn_rl_repo"):
    if _p not in sys.path:
        sys.path.insert(0, _p)

import numpy as np
import ml_dtypes
import concourse.bass as bass
import concourse.bacc as bacc
import concourse.tile as tile
from concourse import mybir
from concourse.masks import make_identity
from concourse.bass_utils import run_bass_kernel_spmd

F32 = mybir.dt.float32
BF16 = mybir.dt.bfloat16
EXP = mybir.ActivationFunctionType.Exp
IDENT = mybir.ActivationFunctionType.Identity
ADD = mybir.AluOpType.add
MULT = mybir.AluOpType.mult

NPBF16 = ml_dtypes.bfloat16

B, S, D = 4, 2048, 1024
H, DK, DV = 16, 64, 64
NCORES = 8
HC = H // 2          # heads per core
HDK = HC * DK        # 512 head dims per core
SQC = 512            # query-chunk width


def build_program(nc: bass.Bass, s=S, d=D, hc=HC):
    hdk = hc * DK
    ck_n = hdk // 128        # proj-col partition-tiles (= head pairs) = 4
    dt_n = d // 128          # D contraction tiles = 8
    skt_n = s // 128         # key tiles = 16
    sq_n = s // SQC          # query chunks = 4
    zn = SQC // 128          # 128-row tiles per query chunk = 4

    xqt = nc.dram_tensor("xqt", [d, s], BF16, kind="ExternalInput")
    xkt = nc.dram_tensor("xkt", [d, s], BF16, kind="ExternalInput")
    xvt = nc.dram_tensor("xvt", [d, s], BF16, kind="ExternalInput")
    wq = nc.dram_tensor("wq", [d, hdk], BF16, kind="ExternalInput")
    wk = nc.dram_tensor("wk", [d, hdk], BF16, kind="ExternalInput")
    wv = nc.dram_tensor("wv", [d, hdk], BF16, kind="ExternalInput")
    bq = nc.dram_tensor("bq", [ck_n, 128, 1], F32, kind="ExternalInput")
    bk = nc.dram_tensor("bk", [ck_n, 128, 1], F32, kind="ExternalInput")
    bv = nc.dram_tensor("bv", [1, hdk], BF16, kind="ExternalInput")
    wo = nc.dram_tensor("wo", [hdk, d], BF16, kind="ExternalInput")
    bo = nc.dram_tensor("bo", [dt_n, 128, 1], F32, kind="ExternalInput")
    msk = nc.dram_tensor("msk", [skt_n, 128, 1], F32, kind="ExternalInput")

    outT_p = nc.dram_tensor("outT_p", [d, s], F32, kind="ExternalOutput")
    wts_p = nc.dram_tensor("wts_p", [s, hdk], F32, kind="ExternalOutput")

    # weights DMA view: rows (q, z, p), cols (pair j, head m, dv)
    wts_v = wts_p.rearrange(
        "(q z p) (j m e) -> q j p m z e", z=zn, p=128, m=2, e=DV
    )

    with tile.TileContext(nc) as tc, \
            tc.tile_pool(name="consts", bufs=1) as consts, \
            tc.tile_pool(name="persist", bufs=1) as persist:
        ident = consts.tile([128, 128], F32, name="ident")
        make_identity(nc, ident)
        ones1 = consts.tile([1, 128], BF16, name="ones1")
        nc.gpsimd.memset(ones1, 1.0)
        msk_sb = consts.tile([128, skt_n], F32, name="msk_sb")
        boT_sb = consts.tile([128, dt_n], F32, name="boT_sb")
        bv_sb = consts.tile([1, hdk], BF16, name="bv_sb")
        bias_t = {}
        for nm in ("q", "k"):
            bias_t[nm] = consts.tile([128, ck_n], F32, name=f"b{nm}_t")

        def load_consts():
            nc.sync.dma_start(out=msk_sb, in_=msk.rearrange("t p one -> p (t one)"))
            nc.sync.dma_start(out=boT_sb, in_=bo.rearrange("t p one -> p (t one)"))
            nc.sync.dma_start(out=bv_sb, in_=bv[:])
            for nm, srct in (("q", bq), ("k", bk)):
                nc.sync.dma_start(
                    out=bias_t[nm], in_=srct.rearrange("t p one -> p (t one)")
                )

        qT = persist.tile([128, ck_n, s], BF16, name="qT")
        kT = persist.tile([128, ck_n, s], BF16, name="kT")
        vtn = persist.tile([128, skt_n, hc, DV + 1], BF16, name="vtn")
        ones_th = consts.tile([128, skt_n * hc], BF16, name="ones_th")
        nc.gpsimd.memset(ones_th, 1.0)
        nc.vector.tensor_copy(
            vtn[:, :, :, DV : DV + 1],
            ones_th.rearrange("p (t h one) -> p t h one", t=skt_n, one=1),
        )
        wo_sb = persist.tile([128, ck_n, d], BF16, name="wo_sb")
        nc.gpsimd.dma_start(out=wo_sb, in_=wo.rearrange("(t p) e -> p t e", p=128))

        ncopy = 0

        with (
            tc.tile_pool(name="xt", bufs=3) as xt_pool,
            tc.tile_pool(name="wz", bufs=3) as wz_pool,
            tc.tile_pool(name="ep", bufs=6) as ep_pool,
            tc.tile_pool(name="ctxu", bufs=3) as ctxu_pool,
            tc.tile_pool(name="wtsT", bufs=2) as wtsT_pool,
            tc.tile_pool(name="wnat", bufs=3) as wnat_pool,
            tc.tile_pool(name="rcp", bufs=3) as rcp_pool,
            tc.tile_pool(name="outsb", bufs=10) as outsb_pool,
            tc.tile_pool(name="sc_ps", bufs=2, space="PSUM") as sc_ps,
            tc.tile_pool(name="ctx_ps", bufs=2, space="PSUM") as ctx_ps,
            tc.tile_pool(name="aux_ps", bufs=2, space="PSUM") as aux_ps,
        ):
            # ---------- phase A helpers ----------
            def load_xt_chunk(xz, sb, width=SQC):
                xt_sb = xt_pool.tile([128, dt_n, width], BF16, name="xt_sb")
                nc.sync.dma_start(
                    out=xt_sb,
                    in_=xz.rearrange("(t p) s -> p t s", p=128)[
                        :, :, sb * width : (sb + 1) * width
                    ],
                )
                return xt_sb

            def proj_qk(w_sb, xt_sb, outT, bt, ck, sb):
                """One [128, 512] chunk of Q^T or K^T."""
                nonlocal ncopy
                pp = aux_ps.tile([128, SQC], F32, name="aux")
                for dt_ in range(dt_n):
                    nc.tensor.matmul(
                        pp,
                        w_sb[:, dt_, ck * 128 : (ck + 1) * 128],
                        xt_sb[:, dt_],
                        start=(dt_ == 0), stop=(dt_ == dt_n - 1),
                    )
                dst = outT[:, ck, sb * SQC : (sb + 1) * SQC]
                if ncopy % 2 == 0:
                    nc.vector.tensor_scalar(
                        out=dst, in0=pp, scalar1=bt[:, ck : ck + 1],
                        scalar2=None, op0=ADD,
                    )
                else:
                    nc.scalar.activation(dst, pp, IDENT, bias=bt[:, ck : ck + 1])
                ncopy += 1

            def proj_v(wv_sb, xt_sb, sb):
                """Four natural-layout [128 s, 512 e] V tiles of chunk sb."""
                nonlocal ncopy
                for stl in range(SQC // 128):
                    vp = aux_ps.tile([128, hdk], F32, name="aux")
                    for dt_ in range(dt_n):
                        nc.tensor.matmul(
                            vp,
                            xt_sb[:, dt_, stl * 128 : (stl + 1) * 128],
                            wv_sb[:, dt_],
                            start=(dt_ == 0), stop=(dt_ == dt_n - 1),
                        )
                    st = sb * (SQC // 128) + stl
                    dst = vtn[:, st, :, 0:DV]
                    srcv = vp.rearrange("p (h e) -> p h e", h=hc)
                    bvv = bvb.rearrange("p (h e) -> p h e", h=hc)
                    if ncopy % 2 == 0:
                        nc.vector.scalar_tensor_tensor(
                            out=dst, in0=vp.rearrange("p (h e) -> p h e", h=hc),
                            scalar=0.0, in1=bvv,
                            op0=mybir.AluOpType.bypass, op1=ADD,
                        )
                    else:
                        nc.vector.scalar_tensor_tensor(
                            out=dst, in0=srcv, scalar=0.0, in1=bvv,
                            op0=mybir.AluOpType.bypass, op1=ADD,
                        )
                    ncopy += 1

            # ---------- phase A: K and V (fully), Q chunk 0 ----------
            wk_sb = wz_pool.tile([128, dt_n, hdk], BF16, name="w_sb")
            nc.gpsimd.dma_start(
                out=wk_sb, in_=wk.rearrange("(t p) e -> p t e", p=128)
            )
            for sb in range(sq_n):
                xt_sb = load_xt_chunk(xkt, sb)
                if sb == 0:
                    load_consts()
                for ck in range(ck_n):
                    proj_qk(wk_sb, xt_sb, kT, bias_t["k"], ck, sb)

            wv_sb = wz_pool.tile([128, dt_n, hdk], BF16, name="w_sb")
            nc.gpsimd.dma_start(
                out=wv_sb, in_=wv.rearrange("(t p) e -> p t e", p=128)
            )
            bvb = persist.tile([128, hdk], BF16, name="bvb")
            pbv = aux_ps.tile([128, hdk], F32, name="aux")
            nc.tensor.matmul(pbv, ones1, bv_sb, start=True, stop=True)
            nc.vector.tensor_copy(bvb, pbv)
            for sb in range(sq_n):
                xt_sb = load_xt_chunk(xvt, sb)
                proj_v(wv_sb, xt_sb, sb)

            wq_sb = wz_pool.tile([128, dt_n, hdk], BF16, name="w_sb")
            nc.gpsimd.dma_start(
                out=wq_sb, in_=wq.rearrange("(t p) e -> p t e", p=128)
            )
            xt_sb0 = load_xt_chunk(xqt, 0)
            for ck in range(ck_n):
                proj_qk(wq_sb, xt_sb0, qT, bias_t["q"], ck, 0)

            # ---------- phase B: attention + deferred post-processing ----
            pending = []

            def qproj_chunk(sb, ck):
                def emit():
                    # xt chunk loaded lazily per (sb): stash on the closure
                    proj_qk(wq_sb, qproj_chunk.xt[sb], qT, bias_t["q"], ck, sb)
                return emit
            qproj_chunk.xt = {}

            def weights_chunk(q, j, m, ctxu, wnat, rc, wtsT_sb):
                def emit():
                    nat = aux_ps.tile([128, zn, DV + 1], F32, name="aux")
                    for zz in range(zn):
                        nc.tensor.transpose(
                            nat[:, zz],
                            ctxu[:, m * SQC + zz * 128 : m * SQC + (zz + 1) * 128],
                            ident[0 : DV + 1, 0 : DV + 1],
                        )
                    nc.vector.reciprocal(rc[:, m], nat[:, :, DV : DV + 1])
                    for zz in range(zn):
                        nc.vector.tensor_scalar(
                            out=wnat[:, m, zz],
                            in0=nat[:, zz, 0:DV],
                            scalar1=rc[:, m, zz],
                            scalar2=None,
                            op0=MULT,
                        )
                    # normalized natural -> head-dim-major (o_proj rhs).
                    wtp = aux_ps.tile([128, zn, 128], F32, name="aux")
                    for zz in range(zn):
                        if m == 0:
                            nc.tensor.transpose(wtp[0:64, zz], wnat[:, 0, zz], ident)
                        else:
                            nc.tensor.matmul(
                                wtp[64:128, zz],
                                wnat[:, 1, zz],
                                ident,
                                start=True, stop=True,
                                tile_position=(0, 64),
                            )
                    nc.sync.dma_start(out=wts_v[q, j, :, m], in_=wnat[:, m])
                    nc.vector.tensor_copy(
                        wtsT_sb[m * 64 : m * 64 + 64, j, :],
                        wtp[m * 64 : m * 64 + 64],
                    )
                return emit

            def oproj_part1(dt_, wtsT_sb, st):
                def emit():
                    op = aux_ps.tile([128, SQC], F32, name="aux")
                    for et in range(ck_n - 1):
                        nc.tensor.matmul(
                            op,
                            wo_sb[:, et, dt_ * 128 : (dt_ + 1) * 128],
                            wtsT_sb[:, et, :],
                            start=(et == 0), stop=(et == ck_n - 2),
                        )
                    out_sb = outsb_pool.tile([128, SQC], F32, name="out_sb")
                    nc.vector.tensor_copy(out_sb, op)
                    st["out_sb"] = out_sb
                return emit

            def oproj_part2(q, dt_, wtsT_sb, st):
                def emit():
                    op = aux_ps.tile([128, SQC], F32, name="aux")
                    nc.tensor.matmul(
                        op,
                        wo_sb[:, ck_n - 1, dt_ * 128 : (dt_ + 1) * 128],
                        wtsT_sb[:, ck_n - 1, :],
                        start=True, stop=True,
                    )
                    out_sb = st["out_sb"]
                    nc.vector.scalar_tensor_tensor(
                        out=out_sb, in0=op, scalar=boT_sb[:, dt_ : dt_ + 1],
                        in1=out_sb, op0=ADD, op1=ADD,
                    )
                    nc.sync.dma_start(
                        out=outT_p[dt_ * 128 : (dt_ + 1) * 128,
                                   q * SQC : (q + 1) * SQC],
                        in_=out_sb,
                    )
                return emit

            def oproj_chunk(q, dt_, wtsT_sb):
                def emit():
                    op = aux_ps.tile([128, SQC], F32, name="aux")
                    for et in range(ck_n):
                        nc.tensor.matmul(
                            op,
                            wo_sb[:, et, dt_ * 128 : (dt_ + 1) * 128],
                            wtsT_sb[:, et, :],
                            start=(et == 0), stop=(et == ck_n - 1),
                        )
                    out_sb = outsb_pool.tile([128, SQC], F32, name="out_sb")
                    nc.vector.tensor_scalar(
                        out=out_sb, in0=op, scalar1=boT_sb[:, dt_ : dt_ + 1],
                        scalar2=None, op0=ADD,
                    )
                    nc.sync.dma_start(
                        out=outT_p[dt_ * 128 : (dt_ + 1) * 128,
                                   q * SQC : (q + 1) * SQC],
                        in_=out_sb,
                    )
                return emit

            for sb in range(1, sq_n):
                def load_next(sb=sb):
                    def emit():
                        qproj_chunk.xt[sb] = load_xt_chunk(xqt, sb)
                    return emit
                pending.append(load_next())
                for ck in range(ck_n):
                    pending.append(qproj_chunk(sb, ck))
            for q in range(sq_n):
                q0 = q * SQC
                wtsT_sb = wtsT_pool.tile([128, ck_n, SQC], BF16, name="wtsT_sb")
                for j in range(ck_n):
                    ctxA = ctx_ps.tile([DV + 1, SQC], F32, name="ctx_t")
                    ctxB = ctx_ps.tile([DV + 1, SQC], F32, name="ctx_t")
                    ctxu = ctxu_pool.tile([DV + 1, 2 * SQC], F32, name="ctxu_t")
                    for t in range(skt_n):
                        sc = sc_ps.tile([128, 2 * SQC], F32, name="sc_t")
                        for m in range(2):
                            lo, hi = m * 64, (m + 1) * 64
                            nc.tensor.matmul(
                                sc[:, m * SQC : (m + 1) * SQC],
                                kT[lo:hi, j, t * 128 : (t + 1) * 128],
                                qT[lo:hi, j, q0 : q0 + SQC],
                                start=True, stop=True,
                                tile_position=(m * 64, 0),
                            )
                        ep = ep_pool.tile([128, 2 * SQC], BF16, name="ep_t")
                        nc.scalar.activation(
                            ep, sc, EXP, bias=msk_sb[:, t : t + 1], scale=0.125
                        )
                        nc.tensor.matmul(
                            ctxA, vtn[:, t, 2 * j], ep[:, 0:SQC],
                            start=(t == 0), stop=(t == skt_n - 1),
                        )
                        if t == skt_n - 1:
                            nc.vector.tensor_copy(ctxu[:, 0:SQC], ctxA)
                        nc.tensor.matmul(
                            ctxB, vtn[:, t, 2 * j + 1], ep[:, SQC : 2 * SQC],
                            start=(t == 0), stop=(t == skt_n - 1),
                        )
                        if t == skt_n - 1:
                            nc.vector.tensor_copy(ctxu[:, SQC : 2 * SQC], ctxB)
                        if pending and t % 2 == 1 and t < 14:
                            pending.pop(0)()
                    wnat = wnat_pool.tile([128, 2, zn, DV], F32, name="wnat_t")
                    rc = rcp_pool.tile([128, 2, zn, 1], F32, name="rc_t")
                    for m in range(2):
                        pending.append(
                            weights_chunk(q, j, m, ctxu, wnat, rc, wtsT_sb)
                        )
                    if q == sq_n - 1 and j == ck_n - 2:
                        oproj_state = [dict() for _ in range(dt_n)]
                        for dt_ in range(dt_n):
                            pending.append(
                                oproj_part1(dt_, wtsT_sb, oproj_state[dt_])
                            )
                if q == sq_n - 1:
                    for dt_ in range(dt_n):
                        pending.append(
                            oproj_part2(q, dt_, wtsT_sb, oproj_state[dt_])
                        )
                else:
                    for dt_ in range(dt_n):
                        pending.append(oproj_chunk(q, dt_, wtsT_sb))
            while pending:
                pending.pop(0)()
    return nc


_CACHE = {}


def _get_program():
    if "nc" not in _CACHE:
        nc = bacc.Bacc("TRN2")
        build_program(nc)
        nc.compile()
        _CACHE["nc"] = nc
    return _CACHE["nc"]


def kernel(query, key, value, mask, Wq, bq, Wk, bk, Wv, bv, Wo, bo, trace=False):
    f32 = lambda a: np.ascontiguousarray(a, dtype=np.float32)
    bf = lambda a: np.ascontiguousarray(np.asarray(a, dtype=np.float32), dtype=NPBF16)
    query, key, value, mask = map(np.asarray, (query, key, value, mask))
    Wq, bq, Wk, bk, Wv, bv, Wo, bo = map(f32, (Wq, bq, Wk, bk, Wv, bv, Wo, bo))
    zeros_bo = np.zeros_like(bo)

    # per-batch transposed bf16 inputs (shared by the two cores of a pair)
    xT = {}
    for b in range(B):
        xT[b] = (
            bf(np.asarray(query[b], np.float32).T),
            bf(np.asarray(key[b], np.float32).T),
            bf(np.asarray(value[b], np.float32).T),
        )

    in_maps = []
    for c in range(NCORES):
        b, g = c // 2, c % 2
        cols = slice(g * HDK, (g + 1) * HDK)
        xq_t, xk_t, xv_t = xT[b]
        in_maps.append({
            "xqt": xq_t, "xkt": xk_t, "xvt": xv_t,
            "wq": bf(Wq[:, cols]), "wk": bf(Wk[:, cols]), "wv": bf(Wv[:, cols]),
            "bq": bq[cols].reshape(HDK // 128, 128, 1),
            "bk": bk[cols].reshape(HDK // 128, 128, 1),
            "bv": bf(bv[cols]).reshape(1, HDK),
            "wo": bf(Wo[cols, :]),
            "bo": (bo if g == 0 else zeros_bo).reshape(D // 128, 128, 1),
            "msk": f32(mask[b, 0, 0]).reshape(S // 128, 128, 1),
        })

    nc = _get_program()
    res = run_bass_kernel_spmd(
        nc, in_maps, core_ids=list(range(NCORES)), trace=trace
    )

    output = np.empty((B, S, D), np.float32)
    weights = np.empty((B, S, H * DV), np.float32)
    for b in range(B):
        output[b] = (res.results[2 * b]["outT_p"] + res.results[2 * b + 1]["outT_p"]).T
        weights[b, :, 0:HDK] = res.results[2 * b]["wts_p"]
        weights[b, :, HDK:] = res.results[2 * b + 1]["wts_p"]
    if trace:
        _CACHE["last_exec_time_ns"] = res.exec_time_ns
        _CACHE["last_res"] = res
    return output, weights


# revision 11
# speedup vs baseline: 1.0228x; 1.0228x over previous
"""Multi-head attention (B=4, S=2048, D=1024, H=16, dk=dv=64) on 8 Trainium2
NeuronCores.

Sharding: core c handles batch b = c//2 and head-group g = c%2 (8 of 16 heads).

Host prep: inputs are transposed (x^T, [D, S]) and cast to bf16 on the host,
weights cast to bf16, so the device does no input transposes at all.

Per core:
  - K^T/Q^T projections [512, 2048] from bf16 x^T (moving) x weight tiles
    (stationary); biases folded into the PSUM->SBUF copies (alternating
    vector/scalar engines). V is projected directly to its natural layout
    [2048 s, 512 e] (x^T tiles as the stationary operand, Wv moving), bias
    added via a K=1 ones-row matmul; a ones column per head is appended so
    the PV matmul also produces softmax row-sums.
  - Attention per head-pair j, 512-wide query chunks: scores^T = K Q^T via
    row-tiled K=64 matmuls, exp on the scalar engine straight out of PSUM
    (mask as per-partition bias, 1/8 scale), bf16 probs, PV accumulated over
    16 key tiles.
  - Context (+rowsum row) is transposed back to natural, normalized with
    reciprocal rowsums, written out as `weights`, then transposed once more
    to head-dim-major (bf16) as the o_proj moving operand.
  - o_proj computed transposed: out^T[d, s] accumulating wo-tiles
    (stationary) x wts^T (moving); bo enters as a per-partition bias on the
    PSUM->SBUF copy (gated to g=0 cores). Host transposes back and sums the
    core pair (the row-parallel all-reduce).
  - Q-projection chunks for query block q+1 and all post-processing run as
    deferred ~1us work units popped inside later attention loops, filling
    the tensor-engine slack under the scalar-engine exp stream.
"""
import sys

for _p in ("/opt/trn_rl_repo", "/root/.axon_site/_ro/trn_rl_repo"):
    if _p not in sys.path:
        sys.path.insert(0, _p)

import numpy as np
import ml_dtypes
import concourse.bass as bass
import concourse.bacc as bacc
import concourse.tile as tile
from concourse import mybir
from concourse.masks import make_identity
from concourse.bass_utils import run_bass_kernel_spmd

F32 = mybir.dt.float32
BF16 = mybir.dt.bfloat16
EXP = mybir.ActivationFunctionType.Exp
IDENT = mybir.ActivationFunctionType.Identity
ADD = mybir.AluOpType.add
MULT = mybir.AluOpType.mult

NPBF16 = ml_dtypes.bfloat16

B, S, D = 4, 2048, 1024
H, DK, DV = 16, 64, 64
NCORES = 8
HC = H // 2          # heads per core
HDK = HC * DK        # 512 head dims per core
SQC = 512            # query-chunk width


def build_program(nc: bass.Bass, s=S, d=D, hc=HC):
    hdk = hc * DK
    ck_n = hdk // 128        # proj-col partition-tiles (= head pairs) = 4
    dt_n = d // 128          # D contraction tiles = 8
    skt_n = s // 128         # key tiles = 16
    sq_n = s // SQC          # query chunks = 4
    zn = SQC // 128          # 128-row tiles per query chunk = 4

    xqt = nc.dram_tensor("xqt", [d, s], BF16, kind="ExternalInput")
    xkt = nc.dram_tensor("xkt", [d, s], BF16, kind="ExternalInput")
    xvt = nc.dram_tensor("xvt", [d, s], BF16, kind="ExternalInput")
    wq = nc.dram_tensor("wq", [d, hdk], BF16, kind="ExternalInput")
    wk = nc.dram_tensor("wk", [d, hdk], BF16, kind="ExternalInput")
    wv = nc.dram_tensor("wv", [d, hdk], BF16, kind="ExternalInput")
    bq = nc.dram_tensor("bq", [ck_n, 128, 1], F32, kind="ExternalInput")
    bk = nc.dram_tensor("bk", [ck_n, 128, 1], F32, kind="ExternalInput")
    bv = nc.dram_tensor("bv", [1, hdk], BF16, kind="ExternalInput")
    wo = nc.dram_tensor("wo", [hdk, d], BF16, kind="ExternalInput")
    bo = nc.dram_tensor("bo", [dt_n, 128, 1], F32, kind="ExternalInput")
    msk = nc.dram_tensor("msk", [skt_n, 128, 1], F32, kind="ExternalInput")

    outT_p = nc.dram_tensor("outT_p", [d, s], F32, kind="ExternalOutput")
    wts_p = nc.dram_tensor("wts_p", [s, hdk], F32, kind="ExternalOutput")

    # weights DMA view: rows (q, z, p), cols (pair j, head m, dv)
    wts_v = wts_p.rearrange(
        "(q z p) (j m e) -> q j p m z e", z=zn, p=128, m=2, e=DV
    )

    with tile.TileContext(nc) as tc, \
            tc.tile_pool(name="consts", bufs=1) as consts, \
            tc.tile_pool(name="persist", bufs=1) as persist:
        ident = consts.tile([128, 128], F32, name="ident")
        make_identity(nc, ident)
        ones1 = consts.tile([1, 128], BF16, name="ones1")
        nc.gpsimd.memset(ones1, 1.0)
        msk_sb = consts.tile([128, skt_n], F32, name="msk_sb")
        boT_sb = consts.tile([128, dt_n], F32, name="boT_sb")
        bv_sb = consts.tile([1, hdk], BF16, name="bv_sb")
        bias_t = {}
        for nm in ("q", "k"):
            bias_t[nm] = consts.tile([128, ck_n], F32, name=f"b{nm}_t")

        def load_consts():
            nc.sync.dma_start(out=msk_sb, in_=msk.rearrange("t p one -> p (t one)"))
            nc.sync.dma_start(out=boT_sb, in_=bo.rearrange("t p one -> p (t one)"))
            nc.sync.dma_start(out=bv_sb, in_=bv[:])
            for nm, srct in (("q", bq), ("k", bk)):
                nc.sync.dma_start(
                    out=bias_t[nm], in_=srct.rearrange("t p one -> p (t one)")
                )

        qT = persist.tile([128, ck_n, s], BF16, name="qT")
        kT = persist.tile([128, ck_n, s], BF16, name="kT")
        vtn = persist.tile([128, skt_n, hc, DV + 1], BF16, name="vtn")
        ones_th = consts.tile([128, skt_n * hc], BF16, name="ones_th")
        nc.gpsimd.memset(ones_th, 1.0)
        nc.vector.tensor_copy(
            vtn[:, :, :, DV : DV + 1],
            ones_th.rearrange("p (t h one) -> p t h one", t=skt_n, one=1),
        )
        wo_sb = persist.tile([128, ck_n, d], BF16, name="wo_sb")
        nc.gpsimd.dma_start(out=wo_sb, in_=wo.rearrange("(t p) e -> p t e", p=128))

        ncopy = 0

        with (
            tc.tile_pool(name="xt", bufs=3) as xt_pool,
            tc.tile_pool(name="wz", bufs=3) as wz_pool,
            tc.tile_pool(name="ep", bufs=6) as ep_pool,
            tc.tile_pool(name="ctxu", bufs=3) as ctxu_pool,
            tc.tile_pool(name="wtsT", bufs=2) as wtsT_pool,
            tc.tile_pool(name="wnat", bufs=3) as wnat_pool,
            tc.tile_pool(name="rcp", bufs=3) as rcp_pool,
            tc.tile_pool(name="outsb", bufs=10) as outsb_pool,
            tc.tile_pool(name="sc_ps", bufs=2, space="PSUM") as sc_ps,
            tc.tile_pool(name="ctx_ps", bufs=2, space="PSUM") as ctx_ps,
            tc.tile_pool(name="aux_ps", bufs=2, space="PSUM") as aux_ps,
        ):
            # ---------- phase A helpers ----------
            def load_xt_chunk(xz, sb, width=SQC):
                xt_sb = xt_pool.tile([128, dt_n, width], BF16, name="xt_sb")
                nc.sync.dma_start(
                    out=xt_sb,
                    in_=xz.rearrange("(t p) s -> p t s", p=128)[
                        :, :, sb * width : (sb + 1) * width
                    ],
                )
                return xt_sb

            def proj_qk(w_sb, xt_sb, outT, bt, ck, sb):
                """One [128, 512] chunk of Q^T or K^T."""
                nonlocal ncopy
                pp = aux_ps.tile([128, SQC], F32, name="aux")
                for dt_ in range(dt_n):
                    nc.tensor.matmul(
                        pp,
                        w_sb[:, dt_, ck * 128 : (ck + 1) * 128],
                        xt_sb[:, dt_],
                        start=(dt_ == 0), stop=(dt_ == dt_n - 1),
                    )
                dst = outT[:, ck, sb * SQC : (sb + 1) * SQC]
                if ncopy % 2 == 0:
                    nc.vector.tensor_scalar(
                        out=dst, in0=pp, scalar1=bt[:, ck : ck + 1],
                        scalar2=None, op0=ADD,
                    )
                else:
                    nc.scalar.activation(dst, pp, IDENT, bias=bt[:, ck : ck + 1])
                ncopy += 1

            def proj_v(wv_sb, xt_sb, sb):
                """Four natural-layout [128 s, 512 e] V tiles of chunk sb."""
                nonlocal ncopy
                for stl in range(SQC // 128):
                    vp = aux_ps.tile([128, hdk], F32, name="aux")
                    for dt_ in range(dt_n):
                        nc.tensor.matmul(
                            vp,
                            xt_sb[:, dt_, stl * 128 : (stl + 1) * 128],
                            wv_sb[:, dt_],
                            start=(dt_ == 0), stop=(dt_ == dt_n - 1),
                        )
                    st = sb * (SQC // 128) + stl
                    dst = vtn[:, st, :, 0:DV]
                    srcv = vp.rearrange("p (h e) -> p h e", h=hc)
                    bvv = bvb.rearrange("p (h e) -> p h e", h=hc)
                    if ncopy % 2 == 0:
                        nc.vector.scalar_tensor_tensor(
                            out=dst, in0=vp.rearrange("p (h e) -> p h e", h=hc),
                            scalar=0.0, in1=bvv,
                            op0=mybir.AluOpType.bypass, op1=ADD,
                        )
                    else:
                        nc.vector.scalar_tensor_tensor(
                            out=dst, in0=srcv, scalar=0.0, in1=bvv,
                            op0=mybir.AluOpType.bypass, op1=ADD,
                        )
                    ncopy += 1

            # ---------- phase A: K and V (fully), Q chunk 0 ----------
            wk_sb = wz_pool.tile([128, dt_n, hdk], BF16, name="w_sb")
            nc.gpsimd.dma_start(
                out=wk_sb, in_=wk.rearrange("(t p) e -> p t e", p=128)
            )
            for sb in range(sq_n):
                xt_sb = load_xt_chunk(xkt, sb)
                if sb == 0:
                    load_consts()
                for ck in range(ck_n):
                    proj_qk(wk_sb, xt_sb, kT, bias_t["k"], ck, sb)

            wv_sb = wz_pool.tile([128, dt_n, hdk], BF16, name="w_sb")
            nc.sync.dma_start(
                out=wv_sb, in_=wv.rearrange("(t p) e -> p t e", p=128)
            )
            bvb = persist.tile([128, hdk], BF16, name="bvb")
            pbv = aux_ps.tile([128, hdk], F32, name="aux")
            nc.tensor.matmul(pbv, ones1, bv_sb, start=True, stop=True)
            nc.vector.tensor_copy(bvb, pbv)
            for sb in range(sq_n):
                xt_sb = load_xt_chunk(xvt, sb)
                proj_v(wv_sb, xt_sb, sb)

            wq_sb = wz_pool.tile([128, dt_n, hdk], BF16, name="w_sb")
            nc.sync.dma_start(
                out=wq_sb, in_=wq.rearrange("(t p) e -> p t e", p=128)
            )
            xt_sb0 = load_xt_chunk(xqt, 0)
            for ck in range(ck_n):
                proj_qk(wq_sb, xt_sb0, qT, bias_t["q"], ck, 0)

            # ---------- phase B: attention + deferred post-processing ----
            pending = []

            def qproj_chunk(sb, ck):
                def emit():
                    # xt chunk loaded lazily per (sb): stash on the closure
                    proj_qk(wq_sb, qproj_chunk.xt[sb], qT, bias_t["q"], ck, sb)
                return emit
            qproj_chunk.xt = {}

            def weights_chunk(q, j, m, ctxu, wnat, rc, wtsT_sb):
                def emit():
                    nat = aux_ps.tile([128, zn, DV + 1], F32, name="aux")
                    for zz in range(zn):
                        nc.tensor.transpose(
                            nat[:, zz],
                            ctxu[:, m * SQC + zz * 128 : m * SQC + (zz + 1) * 128],
                            ident[0 : DV + 1, 0 : DV + 1],
                        )
                    nc.vector.reciprocal(rc[:, m], nat[:, :, DV : DV + 1])
                    for zz in range(zn):
                        nc.vector.tensor_scalar(
                            out=wnat[:, m, zz],
                            in0=nat[:, zz, 0:DV],
                            scalar1=rc[:, m, zz],
                            scalar2=None,
                            op0=MULT,
                        )
                    # normalized natural -> head-dim-major (o_proj rhs).
                    wtp = aux_ps.tile([128, zn, 128], F32, name="aux")
                    for zz in range(zn):
                        if m == 0:
                            nc.tensor.transpose(wtp[0:64, zz], wnat[:, 0, zz], ident)
                        else:
                            nc.tensor.matmul(
                                wtp[64:128, zz],
                                wnat[:, 1, zz],
                                ident,
                                start=True, stop=True,
                                tile_position=(0, 64),
                            )
                    nc.sync.dma_start(out=wts_v[q, j, :, m], in_=wnat[:, m])
                    nc.vector.tensor_copy(
                        wtsT_sb[m * 64 : m * 64 + 64, j, :],
                        wtp[m * 64 : m * 64 + 64],
                    )
                return emit

            def oproj_part1(dt_, wtsT_sb, st):
                def emit():
                    op = aux_ps.tile([128, SQC], F32, name="aux")
                    for et in range(ck_n - 1):
                        nc.tensor.matmul(
                            op,
                            wo_sb[:, et, dt_ * 128 : (dt_ + 1) * 128],
                            wtsT_sb[:, et, :],
                            start=(et == 0), stop=(et == ck_n - 2),
                        )
                    out_sb = outsb_pool.tile([128, SQC], F32, name="out_sb")
                    nc.vector.tensor_copy(out_sb, op)
                    st["out_sb"] = out_sb
                return emit

            def oproj_part2(q, dt_, wtsT_sb, st):
                def emit():
                    op = aux_ps.tile([128, SQC], F32, name="aux")
                    nc.tensor.matmul(
                        op,
                        wo_sb[:, ck_n - 1, dt_ * 128 : (dt_ + 1) * 128],
                        wtsT_sb[:, ck_n - 1, :],
                        start=True, stop=True,
                    )
                    out_sb = st["out_sb"]
                    nc.vector.scalar_tensor_tensor(
                        out=out_sb, in0=op, scalar=boT_sb[:, dt_ : dt_ + 1],
                        in1=out_sb, op0=ADD, op1=ADD,
                    )
                    nc.sync.dma_start(
                        out=outT_p[dt_ * 128 : (dt_ + 1) * 128,
                                   q * SQC : (q + 1) * SQC],
                        in_=out_sb,
                    )
                return emit

            def oproj_chunk(q, dt_, wtsT_sb):
                def emit():
                    op = aux_ps.tile([128, SQC], F32, name="aux")
                    for et in range(ck_n):
                        nc.tensor.matmul(
                            op,
                            wo_sb[:, et, dt_ * 128 : (dt_ + 1) * 128],
                            wtsT_sb[:, et, :],
                            start=(et == 0), stop=(et == ck_n - 1),
                        )
                    out_sb = outsb_pool.tile([128, SQC], F32, name="out_sb")
                    nc.vector.tensor_scalar(
                        out=out_sb, in0=op, scalar1=boT_sb[:, dt_ : dt_ + 1],
                        scalar2=None, op0=ADD,
                    )
                    nc.sync.dma_start(
                        out=outT_p[dt_ * 128 : (dt_ + 1) * 128,
                                   q * SQC : (q + 1) * SQC],
                        in_=out_sb,
                    )
                return emit

            for q in range(sq_n):
                q0 = q * SQC
                if q + 1 < sq_n:
                    def load_next(sb=q + 1):
                        def emit():
                            qproj_chunk.xt[sb] = load_xt_chunk(xqt, sb)
                        return emit
                    pending.append(load_next())
                    for ck in range(ck_n):
                        pending.append(qproj_chunk(q + 1, ck))
                wtsT_sb = wtsT_pool.tile([128, ck_n, SQC], BF16, name="wtsT_sb")
                for j in range(ck_n):
                    ctxA = ctx_ps.tile([DV + 1, SQC], F32, name="ctx_t")
                    ctxB = ctx_ps.tile([DV + 1, SQC], F32, name="ctx_t")
                    ctxu = ctxu_pool.tile([DV + 1, 2 * SQC], F32, name="ctxu_t")
                    for t in range(skt_n):
                        sc = sc_ps.tile([128, 2 * SQC], F32, name="sc_t")
                        for m in range(2):
                            lo, hi = m * 64, (m + 1) * 64
                            nc.tensor.matmul(
                                sc[:, m * SQC : (m + 1) * SQC],
                                kT[lo:hi, j, t * 128 : (t + 1) * 128],
                                qT[lo:hi, j, q0 : q0 + SQC],
                                start=True, stop=True,
                                tile_position=(m * 64, 0),
                            )
                        ep = ep_pool.tile([128, 2 * SQC], BF16, name="ep_t")
                        nc.scalar.activation(
                            ep, sc, EXP, bias=msk_sb[:, t : t + 1], scale=0.125
                        )
                        nc.tensor.matmul(
                            ctxA, vtn[:, t, 2 * j], ep[:, 0:SQC],
                            start=(t == 0), stop=(t == skt_n - 1),
                        )
                        if t == skt_n - 1:
                            nc.vector.tensor_copy(ctxu[:, 0:SQC], ctxA)
                        nc.tensor.matmul(
                            ctxB, vtn[:, t, 2 * j + 1], ep[:, SQC : 2 * SQC],
                            start=(t == 0), stop=(t == skt_n - 1),
                        )
                        if t == skt_n - 1:
                            nc.vector.tensor_copy(ctxu[:, SQC : 2 * SQC], ctxB)
                        if pending and (
                            t % 3 == 2 or (q == sq_n - 1 and t % 2 == 1 and t < 14)
                        ):
                            pending.pop(0)()
                    wnat = wnat_pool.tile([128, 2, zn, DV], F32, name="wnat_t")
                    rc = rcp_pool.tile([128, 2, zn, 1], F32, name="rc_t")
                    for m in range(2):
                        pending.append(
                            weights_chunk(q, j, m, ctxu, wnat, rc, wtsT_sb)
                        )
                    if q == sq_n - 1 and j == ck_n - 2:
                        oproj_state = [dict() for _ in range(dt_n)]
                        for dt_ in range(dt_n):
                            pending.append(
                                oproj_part1(dt_, wtsT_sb, oproj_state[dt_])
                            )
                if q == sq_n - 1:
                    for dt_ in range(dt_n):
                        pending.append(
                            oproj_part2(q, dt_, wtsT_sb, oproj_state[dt_])
                        )
                else:
                    for dt_ in range(dt_n):
                        pending.append(oproj_chunk(q, dt_, wtsT_sb))
            while pending:
                pending.pop(0)()
    return nc


_CACHE = {}


def _get_program():
    if "nc" not in _CACHE:
        nc = bacc.Bacc("TRN2")
        build_program(nc)
        nc.compile()
        _CACHE["nc"] = nc
    return _CACHE["nc"]


def kernel(query, key, value, mask, Wq, bq, Wk, bk, Wv, bv, Wo, bo, trace=False):
    f32 = lambda a: np.ascontiguousarray(a, dtype=np.float32)
    bf = lambda a: np.ascontiguousarray(np.asarray(a, dtype=np.float32), dtype=NPBF16)
    query, key, value, mask = map(np.asarray, (query, key, value, mask))
    Wq, bq, Wk, bk, Wv, bv, Wo, bo = map(f32, (Wq, bq, Wk, bk, Wv, bv, Wo, bo))
    zeros_bo = np.zeros_like(bo)

    # per-batch transposed bf16 inputs (shared by the two cores of a pair)
    xT = {}
    for b in range(B):
        xT[b] = (
            bf(np.asarray(query[b], np.float32).T),
            bf(np.asarray(key[b], np.float32).T),
            bf(np.asarray(value[b], np.float32).T),
        )

    in_maps = []
    for c in range(NCORES):
        b, g = c // 2, c % 2
        cols = slice(g * HDK, (g + 1) * HDK)
        xq_t, xk_t, xv_t = xT[b]
        in_maps.append({
            "xqt": xq_t, "xkt": xk_t, "xvt": xv_t,
            "wq": bf(Wq[:, cols]), "wk": bf(Wk[:, cols]), "wv": bf(Wv[:, cols]),
            "bq": bq[cols].reshape(HDK // 128, 128, 1),
            "bk": bk[cols].reshape(HDK // 128, 128, 1),
            "bv": bf(bv[cols]).reshape(1, HDK),
            "wo": bf(Wo[cols, :]),
            "bo": (bo if g == 0 else zeros_bo).reshape(D // 128, 128, 1),
            "msk": f32(mask[b, 0, 0]).reshape(S // 128, 128, 1),
        })

    nc = _get_program()
    res = run_bass_kernel_spmd(
        nc, in_maps, core_ids=list(range(NCORES)), trace=trace
    )

    output = np.empty((B, S, D), np.float32)
    weights = np.empty((B, S, H * DV), np.float32)
    for b in range(B):
        output[b] = (res.results[2 * b]["outT_p"] + res.results[2 * b + 1]["outT_p"]).T
        weights[b, :, 0:HDK] = res.results[2 * b]["wts_p"]
        weights[b, :, HDK:] = res.results[2 * b + 1]["wts_p"]
    if trace:
        _CACHE["last_exec_time_ns"] = res.exec_time_ns
        _CACHE["last_res"] = res
    return output, weights
